# revision 19
# baseline (speedup 1.0000x reference)
"""Causal self-attention (B=4, T=2048, C=768, H=12) on 8 trn2 NeuronCores.

Sharding: 4 batches x 2 head-groups = 8 cores. Each core computes the qkv
projection + attention for its 6 heads of one batch element in transposed
layout (q^T,k^T as [hd,T], v as [T,hd] -- zero on-device transposes), a
partial output projection over its 384 y-channels for all T, then a pairwise
ReduceScatter sums the two partial projections and hands each core one half
of the rows. All matmuls run in float32r (fast fp32, ~1.5e-4).

Host work is limited to slicing/transposing inputs and restacking outputs.
"""
import numpy as np
from contextlib import ExitStack

import concourse.bass as bass
import concourse.bacc as bacc
import concourse.mybir as mybir
import concourse.tile as tile
from concourse.bass_utils import run_bass_kernel_spmd

B, C, H, HD = 4, 768, 12, 64
N_CORES = 8
LH = H // 2          # local heads per core
F32 = mybir.dt.float32
F32R = mybir.dt.float32r
F16 = mybir.dt.float16
Exp = mybir.ActivationFunctionType.Exp
Copy = mybir.ActivationFunctionType.Copy
PAIRS = [[0, 1], [2, 3], [4, 5], [6, 7]]


def build_program(T=2048, with_bias_qkv=False, with_bias_proj=False, debug=False):
    CK = C // 128            # 6 contract chunks of the hidden dim
    QB = min(512, T)         # query block (free dim of S^T)
    NQB = T // QB
    NKC = T // 128           # key chunks
    DPB = QB // 128          # 128-wide diagonal strips per query block
    QKCOLS = 2 * LH * HD     # 768 local q+k columns
    VCOLS = LH * HD          # 384 local v columns
    WACOLS = QKCOLS + VCOLS  # 1152

    nc = bacc.Bacc("TRN2", target_bir_lowering=False, debug=False,
                   num_devices=N_CORES)
    xT_d = nc.dram_tensor("xT", [C, T], F32R, kind="ExternalInput")
    wa_d = nc.dram_tensor("wa", [C, WACOLS], F32R, kind="ExternalInput")
    wp_d = nc.dram_tensor("wp", [VCOLS, C], F32R, kind="ExternalInput")
    tri_d = nc.dram_tensor("tri", [128, 128], F32R, kind="ExternalInput")
    if with_bias_qkv:
        bq_d = nc.dram_tensor("bqkv", [1, WACOLS], F32R, kind="ExternalInput")
    if with_bias_proj:
        bp_d = nc.dram_tensor("bp", [1, C], F32R, kind="ExternalInput")
    out_d = nc.dram_tensor("out", [T // 2, C], F32, kind="ExternalOutput")
    if debug:
        dbg_v0 = nc.dram_tensor("dbg_v0", [128, LH * (HD + 1)], F32, kind="ExternalOutput")
        dbg_qkT0 = nc.dram_tensor("dbg_qkT0", [128, T], F32, kind="ExternalOutput")
        dbg_kT0 = nc.dram_tensor("dbg_kT0", [128, T], F32, kind="ExternalOutput")
        dbg_yps = nc.dram_tensor("dbg_yps", [HD + 1, 512], F32, kind="ExternalOutput")
        dbg_pt = nc.dram_tensor("dbg_pt", [128, 512], F32, kind="ExternalOutput")
        dbg_rec = nc.dram_tensor("dbg_rec", [1, 512], F32, kind="ExternalOutput")

    with tile.TileContext(nc) as tc, ExitStack() as top:
        persist = top.enter_context(tc.tile_pool(name="persist", bufs=1))
        dram = top.enter_context(tc.tile_pool(name="dram", bufs=1, space="DRAM"))

        # persistent tensors
        qkT = [persist.tile([128, T], F32R, tag=f"qkT{j}", name=f"qkT{j}") for j in range(CK)]
        kTs = [persist.tile([128, T], F32R, tag=f"kTs{j}", name=f"kTs{j}") for j in range(3)]
        yT = [persist.tile([128, T], F32R, tag=f"yT{j}", name=f"yT{j}") for j in range(3)]
        v_sb = [persist.tile([128, LH * (HD + 1)], F32R, tag=f"v{m}", name=f"v{m}")
                for m in range(NKC)]
        tri = persist.tile([128, 128], F32R, tag="tri")
        nc.sync.dma_start(tri[:], tri_d.ap())
        onescol = persist.tile([128, LH], F32R, tag="onescol")
        onescol_f = persist.tile([128, LH], F32, tag="onescol_f")
        nc.vector.memset(onescol_f[:], 1.0)
        nc.vector.tensor_copy(onescol[:], onescol_f[:])
        if with_bias_qkv:
            bq_sb = persist.tile([1, WACOLS], F32R, tag="bq")
            nc.sync.dma_start(bq_sb[:], bq_d.ap())
            onesq = persist.tile([1, QB], F32R, tag="onesq")
            onesq_f = persist.tile([1, QB], F32, tag="onesq_f")
            nc.vector.memset(onesq_f[:], 1.0)
            nc.vector.tensor_copy(onesq[:], onesq_f[:])
        if with_bias_proj:
            bp_sb = persist.tile([1, C], F32R, tag="bp")
            nc.sync.dma_start(bp_sb[:], bp_d.ap())
        if with_bias_qkv or with_bias_proj:
            ones128 = persist.tile([1, 128], F32R, tag="ones128")
            ones128_f = persist.tile([1, 128], F32, tag="ones128_f")
            nc.vector.memset(ones128_f[:], 1.0)
            nc.vector.tensor_copy(ones128[:], ones128_f[:])

        # ---------------- phase A: load + qkv projection ----------------
        with tc.tile_pool(name="phA", bufs=1) as phA, \
             tc.tile_pool(name="psA", bufs=2, space="PSUM") as psA:
            xt = [phA.tile([128, T], F32R, tag=f"xt{i}", name=f"xt{i}") for i in range(CK)]
            wa = [phA.tile([128, WACOLS], F32R, tag=f"wa{i}", name=f"wa{i}") for i in range(CK)]
            for i in range(CK):
                nc.sync.dma_start(xt[i][:], xT_d.ap()[128 * i:128 * (i + 1), :])
                nc.sync.dma_start(wa[i][:], wa_d.ap()[128 * i:128 * (i + 1), :])

            # Emission follows consumption order of phase B: for each query
            # block, its qk chains (+ per-slice kTs swaps) then its v chunks.
            def v_chain(m):
                vps = psA.tile([128, VCOLS], F32, tag="vps", name=f"vps{m}")
                for i in range(CK):
                    nc.tensor.matmul(
                        vps[:], xt[i][:, 128 * m:128 * (m + 1)],
                        wa[i][:, QKCOLS:WACOLS],
                        start=(i == 0),
                        stop=(i == CK - 1 and not with_bias_qkv))
                if with_bias_qkv:
                    nc.tensor.matmul(vps[:], ones128[:],
                                     bq_sb[:, QKCOLS:WACOLS],
                                     start=False, stop=True)
                nc.vector.tensor_copy(
                    v_sb[m][:].rearrange("p (h c) -> p h c", c=HD + 1)[:, :, 0:HD],
                    vps[:].rearrange("p (h c) -> p h c", c=HD))
                nc.vector.tensor_copy(
                    v_sb[m][:].rearrange("p (h c) -> p h c", c=HD + 1)[:, :, HD:HD + 1],
                    onescol[:].rearrange("p (h c) -> p h c", c=1))

            for n in range(NQB):
                for j in range(CK):
                    qps = psA.tile([128, QB], F32, tag="qps", name=f"qps{j}_{n}")
                    for i in range(CK):
                        nc.tensor.matmul(
                            qps[:], wa[i][:, 128 * j:128 * (j + 1)],
                            xt[i][:, QB * n:QB * (n + 1)],
                            start=(i == 0),
                            stop=(i == CK - 1 and not with_bias_qkv))
                    if with_bias_qkv:
                        nc.tensor.matmul(
                            qps[:], bq_sb[:, 128 * j:128 * (j + 1)], onesq[:],
                            start=False, stop=True)
                    nc.vector.tensor_copy(qkT[j][:, QB * n:QB * (n + 1)], qps[:])
                for m in range(DPB * n, DPB * (n + 1)):
                    v_chain(m)

        if debug:
            dv = persist.tile([128, LH * (HD + 1)], F32, tag="dv")
            nc.vector.tensor_copy(dv[:], v_sb[0][:])
            nc.sync.dma_start(dbg_v0.ap(), dv[:])
            dq = persist.tile([128, T], F32, tag="dq")
            nc.vector.tensor_copy(dq[:], qkT[0][:])
            nc.sync.dma_start(dbg_qkT0.ap(), dq[:])
            dk = persist.tile([128, T], F32, tag="dk")
            nc.vector.tensor_copy(dk[:], qkT[3][:])
            nc.sync.dma_start(dbg_kT0.ap(), dk[:])

        if debug:
            dv = persist.tile([128, LH * (HD + 1)], F32, tag="dv")
            nc.vector.tensor_copy(dv[:], v_sb[0][:])
            nc.sync.dma_start(dbg_v0.ap(), dv[:])
            dq = persist.tile([128, T], F32, tag="dq")
            nc.vector.tensor_copy(dq[:], qkT[0][:])
            nc.sync.dma_start(dbg_qkT0.ap(), dq[:])
            dk = persist.tile([128, T], F32, tag="dk")
            nc.vector.tensor_copy(dk[:], qkT[3][:])
            nc.sync.dma_start(dbg_kT0.ap(), dk[:])

        # half-swapped copies of k^T so both PE row groups can host any head
        for j in range(3):
            nc.sync.dma_start(kTs[j][64:128, :], qkT[3 + j][0:64, :])
            nc.sync.dma_start(kTs[j][0:64, :], qkT[3 + j][64:128, :])

        # ---------------- phase B+C interleaved ----------------
        # n-outer: once query block n is done for all heads, its four output
        # rows chunks are projected immediately; each half's ReduceScatter is
        # issued as soon as its projections are stored, hiding the collective
        # under the remaining attention work.
        partial = dram.tile([T, C], F32)
        rs_out = dram.tile([T // 2, C], F32)
        with tc.tile_pool(name="phB", bufs=6) as phB, \
             tc.tile_pool(name="phBs", bufs=2) as phBs, \
             tc.tile_pool(name="phC", bufs=1) as phC, \
             tc.tile_pool(name="stg", bufs=3) as stg, \
             tc.tile_pool(name="psS", bufs=3, space="PSUM") as psS, \
             tc.tile_pool(name="psY", bufs=2, space="PSUM") as psY, \
             tc.tile_pool(name="psC", bufs=1, space="PSUM") as psC:
            wp = [phC.tile([128, C], F32R, tag=f"wp{j}", name=f"wp{j}") for j in range(3)]
            for j in range(3):
                nc.sync.dma_start(wp[j][:], wp_d.ap()[128 * j:128 * (j + 1), :])

            for n in range(NQB):
                nkc = (QB // 128) * (n + 1)
                for h in range(LH):
                    jq, rq = h // 2, 64 * (h % 2)
                    klo = qkT[3 + jq] if h % 2 == 0 else kTs[jq]
                    khi = kTs[jq] if h % 2 == 0 else qkT[3 + jq]
                    yps = psY.tile([HD + 1, QB], F32, tag="yps")
                    # stage this (head, block)'s q at the opposite base
                    ob = 64 - rq
                    qst = phB.tile([128, QB], F32R, tag="qst", bufs=2)
                    nc.sync.dma_start(
                        qst[ob:ob + 64, :],
                        qkT[jq][rq:rq + 64, QB * n:QB * (n + 1)])
                    for kc0 in range(0, nkc, 2):
                        pair = [kc0] if kc0 + 1 >= nkc else [kc0, kc0 + 1]
                        # both S^T tiles of the pair land in one 2-bank psum
                        # tile; adjacent matmuls in distinct PE row groups run
                        # concurrently (K=64 row tiling)
                        spw = psS.tile([128, 2 * QB], F32, tag="sps", bufs=2)
                        ptw = phB.tile([128, 2 * QB], F32R, tag="pt", bufs=4)
                        offs = []
                        for pi, kc in enumerate(pair):
                            d = kc - DPB * n
                            c0 = 128 * d if d > 0 else 0
                            off = pi * QB
                            offs.append((kc, d, c0, off))
                            kt, rb = (klo, 0) if kc % 2 == 0 else (khi, 64)
                            if rb == rq:
                                qt_ap = qkT[jq][rq:rq + 64,
                                                QB * n + c0:QB * (n + 1)]
                            else:
                                qt_ap = qst[ob:ob + 64, c0:QB]
                            nc.tensor.matmul(
                                spw[:, off + c0:off + QB],
                                kt[rb:rb + 64, 128 * kc:128 * (kc + 1)],
                                qt_ap, start=True, stop=True)
                        if len(pair) == 2 and all(d < 0 for _, d, _, _ in offs):
                            nc.scalar.activation(ptw[:], spw[:], Exp, scale=0.125)
                        else:
                            for kc, d, c0, off in offs:
                                nc.scalar.activation(
                                    ptw[:, off + c0:off + QB],
                                    spw[:, off + c0:off + QB],
                                    Exp, scale=0.125)
                        for kc, d, c0, off in offs:
                            if d >= 0:
                                nc.vector.tensor_tensor(
                                    ptw[:, off + c0:off + c0 + 128],
                                    ptw[:, off + c0:off + c0 + 128],
                                    tri[:], mybir.AluOpType.mult)
                            if debug and h == 0 and n == 0 and kc == 0:
                                dpt = phB.tile([128, QB], F32, tag="dpt")
                                nc.vector.tensor_copy(dpt[:], ptw[:, 0:QB])
                                nc.sync.dma_start(dbg_pt.ap(), dpt[:, 0:512])
                        for kc, d, c0, off in offs:
                            nc.tensor.matmul(
                                yps[:, c0:QB],
                                v_sb[kc][:, (HD + 1) * h:(HD + 1) * (h + 1)],
                                ptw[:, off + c0:off + QB],
                                start=(kc == 0), stop=(kc == nkc - 1))
                    if debug and h == 0 and n == 0:
                        dyp = phBs.tile([HD + 1, QB], F32, tag="dyp")
                        nc.vector.tensor_copy(dyp[:], yps[:])
                        nc.sync.dma_start(dbg_yps.ap(), dyp[:, 0:512])
                    # normalize: yT[.] = yps[0:64] / yps[64]
                    s_sb = phBs.tile([1, QB], F32, tag="s_sb")
                    nc.vector.tensor_copy(s_sb[:], yps[HD:HD + 1, :])
                    rec = phBs.tile([1, QB], F32, tag="rec")
                    scr = phBs.tile([1, QB], F32, tag="scr")
                    nc.vector.reciprocal_approx_accurate(rec[:], s_sb[:], scr[:])
                    if debug and h == 0 and n == 0:
                        nc.sync.dma_start(dbg_rec.ap(), rec[:, 0:512])
                    recb = phBs.tile([64, QB], F32, tag="recb")
                    nc.gpsimd.partition_broadcast(recb[:], rec[:])
                    nc.vector.tensor_tensor(
                        yT[jq][rq:rq + 64, QB * n:QB * (n + 1)],
                        yps[0:HD, :], recb[:], mybir.AluOpType.mult)

                # project this query block's row chunks
                for m in range(DPB * n, DPB * (n + 1)):
                    pps = psC.tile([128, C], F32, tag="pp")
                    for ncol, c0, c1 in ((0, 0, 512), (1, 512, C)):
                        for j in range(3):
                            nc.tensor.matmul(
                                pps[:, c0:c1],
                                yT[j][:, 128 * m:128 * (m + 1)],
                                wp[j][:, c0:c1],
                                start=(j == 0),
                                stop=(j == 2 and not with_bias_proj))
                        if with_bias_proj:
                            nc.tensor.matmul(pps[:, c0:c1], ones128[:],
                                             bp_sb[:, c0:c1],
                                             start=False, stop=True)
                    ost = stg.tile([128, C], F32, tag="ost")
                    nc.scalar.activation(ost[:], pps[:], Copy)
                    nc.sync.dma_start(
                        partial[128 * m:128 * (m + 1), :], ost[:])
            nc.gpsimd.collective_compute(
                "ReduceScatter", mybir.AluOpType.add,
                replica_groups=PAIRS,
                ins=[partial.opt()], outs=[rs_out.opt()])
            nc.sync.dma_start(out_d.ap(), rs_out[:])
    nc.compile()
    return nc


def shard_inputs(x, W_attn, b_attn, W_proj, b_proj):
    """Per-core input maps. Core c = 2*b + g handles batch b, head-group g."""
    T = x.shape[1]
    tri = np.tril(np.ones((128, 128), dtype=np.float32)).T.copy()
    # tri[k_row, q_col] = 1 where k <= q  (lower-tri in (q,k) = upper in (k,q))
    with_bias_qkv = bool(np.any(b_attn))
    with_bias_proj = bool(np.any(b_proj))
    in_maps = []
    for c in range(N_CORES):
        b, g = divmod(c, 2)
        xT = np.ascontiguousarray(x[b].T)
        wq = W_attn[:, 384 * g:384 * (g + 1)]
        wk = W_attn[:, C + 384 * g:C + 384 * (g + 1)]
        wv = W_attn[:, 2 * C + 384 * g:2 * C + 384 * (g + 1)]
        wa = np.ascontiguousarray(np.concatenate([wq, wk, wv], axis=1))
        wp = np.ascontiguousarray(W_proj[384 * g:384 * (g + 1), :])
        m = {"xT": xT, "wa": wa, "wp": wp, "tri": tri}
        if with_bias_qkv:
            m["bqkv"] = np.concatenate(
                [b_attn[384 * g:384 * (g + 1)],
                 b_attn[C + 384 * g:C + 384 * (g + 1)],
                 b_attn[2 * C + 384 * g:2 * C + 384 * (g + 1)]]
            ).reshape(1, -1).astype(np.float32)
        if with_bias_proj:
            m["bp"] = (b_proj / 2.0).reshape(1, -1).astype(np.float32)
        in_maps.append(m)
    return in_maps, with_bias_qkv, with_bias_proj


def unshard_output(results, T):
    out = np.empty((B, T, C), dtype=np.float32)
    for b in range(B):
        out[b, :T // 2] = results[2 * b]["out"]
        out[b, T // 2:] = results[2 * b + 1]["out"]
    return out


_CACHED = {}


def kernel(x, W_attn, b_attn, W_proj, b_proj):
    x = np.asarray(x, dtype=np.float32)
    W_attn = np.asarray(W_attn, dtype=np.float32)
    b_attn = np.asarray(b_attn, dtype=np.float32)
    W_proj = np.asarray(W_proj, dtype=np.float32)
    b_proj = np.asarray(b_proj, dtype=np.float32)
    T = x.shape[1]
    in_maps, wbq, wbp = shard_inputs(x, W_attn, b_attn, W_proj, b_proj)
    key = (T, wbq, wbp)
    if key not in _CACHED:
        _CACHED[key] = build_program(T, wbq, wbp)
    nc = _CACHED[key]
    res = run_bass_kernel_spmd(nc, in_maps, list(range(N_CORES)))
    return unshard_output(res.results, T)


# revision 20
# speedup vs baseline: 1.0900x; 1.0900x over previous
"""Causal self-attention (B=4, T=2048, C=768, H=12) on 8 trn2 NeuronCores.

Sharding: 4 batches x 2 head-groups = 8 cores. Each core computes the qkv
projection + attention for its 6 heads of one batch element in transposed
layout (q^T,k^T as [hd,T], v as [T,hd] -- zero on-device transposes), a
partial output projection over its 384 y-channels for all T, then a pairwise
ReduceScatter sums the two partial projections and hands each core one half
of the rows. All matmuls run in float32r (fast fp32, ~1.5e-4).

Host work is limited to slicing/transposing inputs and restacking outputs.
"""
import numpy as np
from contextlib import ExitStack

import concourse.bass as bass
import concourse.bacc as bacc
import concourse.mybir as mybir
import concourse.tile as tile
from concourse.bass_utils import run_bass_kernel_spmd

B, C, H, HD = 4, 768, 12, 64
N_CORES = 8
LH = H // 2          # local heads per core
F32 = mybir.dt.float32
F32R = mybir.dt.float32r
F16 = mybir.dt.float16
Exp = mybir.ActivationFunctionType.Exp
Copy = mybir.ActivationFunctionType.Copy
PAIRS = [[0, 1], [2, 3], [4, 5], [6, 7]]


def build_program(T=2048, with_bias_qkv=False, with_bias_proj=False, debug=False):
    CK = C // 128            # 6 contract chunks of the hidden dim
    QB = min(512, T)         # query block (free dim of S^T)
    NQB = T // QB
    NKC = T // 128           # key chunks
    DPB = QB // 128          # 128-wide diagonal strips per query block
    QKCOLS = 2 * LH * HD     # 768 local q+k columns
    VCOLS = LH * HD          # 384 local v columns
    WACOLS = QKCOLS + VCOLS  # 1152

    nc = bacc.Bacc("TRN2", target_bir_lowering=False, debug=False,
                   num_devices=N_CORES)
    xT_d = nc.dram_tensor("xT", [C, T], F32R, kind="ExternalInput")
    wa_d = nc.dram_tensor("wa", [C, WACOLS], F32R, kind="ExternalInput")
    wp_d = nc.dram_tensor("wp", [VCOLS, C], F32R, kind="ExternalInput")
    tri_d = nc.dram_tensor("tri", [128, 128], F32R, kind="ExternalInput")
    if with_bias_qkv:
        bq_d = nc.dram_tensor("bqkv", [1, WACOLS], F32R, kind="ExternalInput")
    if with_bias_proj:
        bp_d = nc.dram_tensor("bp", [1, C], F32R, kind="ExternalInput")
    out_d = nc.dram_tensor("out", [T // 2, C], F32, kind="ExternalOutput")
    if debug:
        dbg_v0 = nc.dram_tensor("dbg_v0", [128, LH * (HD + 1)], F32, kind="ExternalOutput")
        dbg_qkT0 = nc.dram_tensor("dbg_qkT0", [128, T], F32, kind="ExternalOutput")
        dbg_kT0 = nc.dram_tensor("dbg_kT0", [128, T], F32, kind="ExternalOutput")
        dbg_yps = nc.dram_tensor("dbg_yps", [HD + 1, 512], F32, kind="ExternalOutput")
        dbg_pt = nc.dram_tensor("dbg_pt", [128, 512], F32, kind="ExternalOutput")
        dbg_rec = nc.dram_tensor("dbg_rec", [1, 512], F32, kind="ExternalOutput")

    with tile.TileContext(nc) as tc, ExitStack() as top:
        persist = top.enter_context(tc.tile_pool(name="persist", bufs=1))
        dram = top.enter_context(tc.tile_pool(name="dram", bufs=1, space="DRAM"))

        # persistent tensors
        qkT = [persist.tile([128, T], F32R, tag=f"qkT{j}", name=f"qkT{j}") for j in range(CK)]
        kTs = [persist.tile([128, T], F32R, tag=f"kTs{j}", name=f"kTs{j}") for j in range(3)]
        yT = [persist.tile([128, T], F32R, tag=f"yT{j}", name=f"yT{j}") for j in range(3)]
        v_sb = [persist.tile([128, LH * (HD + 1)], F32R, tag=f"v{m}", name=f"v{m}")
                for m in range(NKC)]
        tri = persist.tile([128, 128], F32R, tag="tri")
        nc.sync.dma_start(tri[:], tri_d.ap())
        onescol = persist.tile([128, LH], F32R, tag="onescol")
        onescol_f = persist.tile([128, LH], F32, tag="onescol_f")
        nc.vector.memset(onescol_f[:], 1.0)
        nc.vector.tensor_copy(onescol[:], onescol_f[:])
        if with_bias_qkv:
            bq_sb = persist.tile([1, WACOLS], F32R, tag="bq")
            nc.sync.dma_start(bq_sb[:], bq_d.ap())
            onesq = persist.tile([1, QB], F32R, tag="onesq")
            onesq_f = persist.tile([1, QB], F32, tag="onesq_f")
            nc.vector.memset(onesq_f[:], 1.0)
            nc.vector.tensor_copy(onesq[:], onesq_f[:])
        if with_bias_proj:
            bp_sb = persist.tile([1, C], F32R, tag="bp")
            nc.sync.dma_start(bp_sb[:], bp_d.ap())
        if with_bias_qkv or with_bias_proj:
            ones128 = persist.tile([1, 128], F32R, tag="ones128")
            ones128_f = persist.tile([1, 128], F32, tag="ones128_f")
            nc.vector.memset(ones128_f[:], 1.0)
            nc.vector.tensor_copy(ones128[:], ones128_f[:])

        # ---------------- phase A: load + qkv projection ----------------
        with tc.tile_pool(name="phA", bufs=1) as phA, \
             tc.tile_pool(name="psA", bufs=2, space="PSUM") as psA:
            xt = [phA.tile([128, T], F32R, tag=f"xt{i}", name=f"xt{i}") for i in range(CK)]
            wa = [phA.tile([128, WACOLS], F32R, tag=f"wa{i}", name=f"wa{i}") for i in range(CK)]
            for i in range(CK):
                nc.sync.dma_start(xt[i][:], xT_d.ap()[128 * i:128 * (i + 1), :])
                nc.sync.dma_start(wa[i][:], wa_d.ap()[128 * i:128 * (i + 1), :])

            # Emission follows consumption order of phase B: for each query
            # block, its qk chains (+ per-slice kTs swaps) then its v chunks.
            def v_chain(m):
                vps = psA.tile([128, VCOLS], F32, tag="vps", name=f"vps{m}")
                for i in range(CK):
                    nc.tensor.matmul(
                        vps[:], xt[i][:, 128 * m:128 * (m + 1)],
                        wa[i][:, QKCOLS:WACOLS],
                        start=(i == 0),
                        stop=(i == CK - 1 and not with_bias_qkv))
                if with_bias_qkv:
                    nc.tensor.matmul(vps[:], ones128[:],
                                     bq_sb[:, QKCOLS:WACOLS],
                                     start=False, stop=True)
                nc.vector.tensor_copy(
                    v_sb[m][:].rearrange("p (h c) -> p h c", c=HD + 1)[:, :, 0:HD],
                    vps[:].rearrange("p (h c) -> p h c", c=HD))
                nc.vector.tensor_copy(
                    v_sb[m][:].rearrange("p (h c) -> p h c", c=HD + 1)[:, :, HD:HD + 1],
                    onescol[:].rearrange("p (h c) -> p h c", c=1))

            for n in range(NQB):
                for j in range(CK):
                    qps = psA.tile([128, QB], F32, tag="qps", name=f"qps{j}_{n}")
                    for i in range(CK):
                        nc.tensor.matmul(
                            qps[:], wa[i][:, 128 * j:128 * (j + 1)],
                            xt[i][:, QB * n:QB * (n + 1)],
                            start=(i == 0),
                            stop=(i == CK - 1 and not with_bias_qkv))
                    if with_bias_qkv:
                        nc.tensor.matmul(
                            qps[:], bq_sb[:, 128 * j:128 * (j + 1)], onesq[:],
                            start=False, stop=True)
                    nc.vector.tensor_copy(qkT[j][:, QB * n:QB * (n + 1)], qps[:])
                for m in range(DPB * n, DPB * (n + 1)):
                    v_chain(m)

        if debug:
            dv = persist.tile([128, LH * (HD + 1)], F32, tag="dv")
            nc.vector.tensor_copy(dv[:], v_sb[0][:])
            nc.sync.dma_start(dbg_v0.ap(), dv[:])
            dq = persist.tile([128, T], F32, tag="dq")
            nc.vector.tensor_copy(dq[:], qkT[0][:])
            nc.sync.dma_start(dbg_qkT0.ap(), dq[:])
            dk = persist.tile([128, T], F32, tag="dk")
            nc.vector.tensor_copy(dk[:], qkT[3][:])
            nc.sync.dma_start(dbg_kT0.ap(), dk[:])

        if debug:
            dv = persist.tile([128, LH * (HD + 1)], F32, tag="dv")
            nc.vector.tensor_copy(dv[:], v_sb[0][:])
            nc.sync.dma_start(dbg_v0.ap(), dv[:])
            dq = persist.tile([128, T], F32, tag="dq")
            nc.vector.tensor_copy(dq[:], qkT[0][:])
            nc.sync.dma_start(dbg_qkT0.ap(), dq[:])
            dk = persist.tile([128, T], F32, tag="dk")
            nc.vector.tensor_copy(dk[:], qkT[3][:])
            nc.sync.dma_start(dbg_kT0.ap(), dk[:])

        # half-swapped copies of k^T so both PE row groups can host any head
        for j in range(3):
            nc.sync.dma_start(kTs[j][64:128, :], qkT[3 + j][0:64, :])
            nc.sync.dma_start(kTs[j][0:64, :], qkT[3 + j][64:128, :])

        # ---------------- phase B+C interleaved ----------------
        # n-outer: once query block n is done for all heads, its four output
        # rows chunks are projected immediately; each half's ReduceScatter is
        # issued as soon as its projections are stored, hiding the collective
        # under the remaining attention work.
        # fp16 exchange: halves ReduceScatter bytes (~30us); partial values
        # are O(1) so fp16's 11-bit mantissa costs ~3e-4, no overflow risk
        partial = dram.tile([T, C], F16)
        rs_out = dram.tile([T // 2, C], F16)
        with tc.tile_pool(name="phB", bufs=6) as phB, \
             tc.tile_pool(name="phBs", bufs=2) as phBs, \
             tc.tile_pool(name="phC", bufs=1) as phC, \
             tc.tile_pool(name="stg", bufs=3) as stg, \
             tc.tile_pool(name="psS", bufs=3, space="PSUM") as psS, \
             tc.tile_pool(name="psY", bufs=2, space="PSUM") as psY, \
             tc.tile_pool(name="psC", bufs=1, space="PSUM") as psC:
            wp = [phC.tile([128, C], F32R, tag=f"wp{j}", name=f"wp{j}") for j in range(3)]
            for j in range(3):
                nc.sync.dma_start(wp[j][:], wp_d.ap()[128 * j:128 * (j + 1), :])

            for n in range(NQB):
                nkc = (QB // 128) * (n + 1)
                for h in range(LH):
                    jq, rq = h // 2, 64 * (h % 2)
                    klo = qkT[3 + jq] if h % 2 == 0 else kTs[jq]
                    khi = kTs[jq] if h % 2 == 0 else qkT[3 + jq]
                    yps = psY.tile([HD + 1, QB], F32, tag="yps")
                    # stage this (head, block)'s q at the opposite base
                    ob = 64 - rq
                    qst = phB.tile([128, QB], F32R, tag="qst", bufs=2)
                    nc.sync.dma_start(
                        qst[ob:ob + 64, :],
                        qkT[jq][rq:rq + 64, QB * n:QB * (n + 1)])
                    for kc0 in range(0, nkc, 2):
                        pair = [kc0] if kc0 + 1 >= nkc else [kc0, kc0 + 1]
                        # both S^T tiles of the pair land in one 2-bank psum
                        # tile; adjacent matmuls in distinct PE row groups run
                        # concurrently (K=64 row tiling)
                        spw = psS.tile([128, 2 * QB], F32, tag="sps", bufs=2)
                        ptw = phB.tile([128, 2 * QB], F32R, tag="pt", bufs=4)
                        offs = []
                        for pi, kc in enumerate(pair):
                            d = kc - DPB * n
                            c0 = 128 * d if d > 0 else 0
                            off = pi * QB
                            offs.append((kc, d, c0, off))
                            kt, rb = (klo, 0) if kc % 2 == 0 else (khi, 64)
                            if rb == rq:
                                qt_ap = qkT[jq][rq:rq + 64,
                                                QB * n + c0:QB * (n + 1)]
                            else:
                                qt_ap = qst[ob:ob + 64, c0:QB]
                            nc.tensor.matmul(
                                spw[:, off + c0:off + QB],
                                kt[rb:rb + 64, 128 * kc:128 * (kc + 1)],
                                qt_ap, start=True, stop=True)
                        if len(pair) == 2 and all(d < 0 for _, d, _, _ in offs):
                            nc.scalar.activation(ptw[:], spw[:], Exp, scale=0.125)
                        else:
                            for kc, d, c0, off in offs:
                                nc.scalar.activation(
                                    ptw[:, off + c0:off + QB],
                                    spw[:, off + c0:off + QB],
                                    Exp, scale=0.125)
                        for kc, d, c0, off in offs:
                            if d >= 0:
                                nc.vector.tensor_tensor(
                                    ptw[:, off + c0:off + c0 + 128],
                                    ptw[:, off + c0:off + c0 + 128],
                                    tri[:], mybir.AluOpType.mult)
                            if debug and h == 0 and n == 0 and kc == 0:
                                dpt = phB.tile([128, QB], F32, tag="dpt")
                                nc.vector.tensor_copy(dpt[:], ptw[:, 0:QB])
                                nc.sync.dma_start(dbg_pt.ap(), dpt[:, 0:512])
                        for kc, d, c0, off in offs:
                            nc.tensor.matmul(
                                yps[:, c0:QB],
                                v_sb[kc][:, (HD + 1) * h:(HD + 1) * (h + 1)],
                                ptw[:, off + c0:off + QB],
                                start=(kc == 0), stop=(kc == nkc - 1))
                    if debug and h == 0 and n == 0:
                        dyp = phBs.tile([HD + 1, QB], F32, tag="dyp")
                        nc.vector.tensor_copy(dyp[:], yps[:])
                        nc.sync.dma_start(dbg_yps.ap(), dyp[:, 0:512])
                    # normalize: yT[.] = yps[0:64] / yps[64]
                    s_sb = phBs.tile([1, QB], F32, tag="s_sb")
                    nc.vector.tensor_copy(s_sb[:], yps[HD:HD + 1, :])
                    rec = phBs.tile([1, QB], F32, tag="rec")
                    scr = phBs.tile([1, QB], F32, tag="scr")
                    nc.vector.reciprocal_approx_accurate(rec[:], s_sb[:], scr[:])
                    if debug and h == 0 and n == 0:
                        nc.sync.dma_start(dbg_rec.ap(), rec[:, 0:512])
                    recb = phBs.tile([64, QB], F32, tag="recb")
                    nc.gpsimd.partition_broadcast(recb[:], rec[:])
                    nc.vector.tensor_tensor(
                        yT[jq][rq:rq + 64, QB * n:QB * (n + 1)],
                        yps[0:HD, :], recb[:], mybir.AluOpType.mult)

                # project this query block's row chunks
                for m in range(DPB * n, DPB * (n + 1)):
                    pps = psC.tile([128, C], F32, tag="pp")
                    for ncol, c0, c1 in ((0, 0, 512), (1, 512, C)):
                        for j in range(3):
                            nc.tensor.matmul(
                                pps[:, c0:c1],
                                yT[j][:, 128 * m:128 * (m + 1)],
                                wp[j][:, c0:c1],
                                start=(j == 0),
                                stop=(j == 2 and not with_bias_proj))
                        if with_bias_proj:
                            nc.tensor.matmul(pps[:, c0:c1], ones128[:],
                                             bp_sb[:, c0:c1],
                                             start=False, stop=True)
                    ost = stg.tile([128, C], F16, tag="ost")
                    nc.scalar.activation(ost[:], pps[:], Copy)
                    nc.sync.dma_start(
                        partial[128 * m:128 * (m + 1), :], ost[:])
            nc.gpsimd.collective_compute(
                "ReduceScatter", mybir.AluOpType.add,
                replica_groups=PAIRS,
                ins=[partial.opt()], outs=[rs_out.opt()])
            for q4 in range(4):
                r0, r1 = q4 * T // 8, (q4 + 1) * T // 8
                nc.gpsimd.dma_start(out=out_d.ap()[r0:r1, :],
                                    in_=rs_out[r0:r1, :])
    nc.compile()
    return nc


def shard_inputs(x, W_attn, b_attn, W_proj, b_proj):
    """Per-core input maps. Core c = 2*b + g handles batch b, head-group g."""
    T = x.shape[1]
    tri = np.tril(np.ones((128, 128), dtype=np.float32)).T.copy()
    # tri[k_row, q_col] = 1 where k <= q  (lower-tri in (q,k) = upper in (k,q))
    with_bias_qkv = bool(np.any(b_attn))
    with_bias_proj = bool(np.any(b_proj))
    in_maps = []
    for c in range(N_CORES):
        b, g = divmod(c, 2)
        xT = np.ascontiguousarray(x[b].T)
        wq = W_attn[:, 384 * g:384 * (g + 1)]
        wk = W_attn[:, C + 384 * g:C + 384 * (g + 1)]
        wv = W_attn[:, 2 * C + 384 * g:2 * C + 384 * (g + 1)]
        wa = np.ascontiguousarray(np.concatenate([wq, wk, wv], axis=1))
        wp = np.ascontiguousarray(W_proj[384 * g:384 * (g + 1), :])
        m = {"xT": xT, "wa": wa, "wp": wp, "tri": tri}
        if with_bias_qkv:
            m["bqkv"] = np.concatenate(
                [b_attn[384 * g:384 * (g + 1)],
                 b_attn[C + 384 * g:C + 384 * (g + 1)],
                 b_attn[2 * C + 384 * g:2 * C + 384 * (g + 1)]]
            ).reshape(1, -1).astype(np.float32)
        if with_bias_proj:
            m["bp"] = (b_proj / 2.0).reshape(1, -1).astype(np.float32)
        in_maps.append(m)
    return in_maps, with_bias_qkv, with_bias_proj


def unshard_output(results, T):
    out = np.empty((B, T, C), dtype=np.float32)
    for b in range(B):
        out[b, :T // 2] = results[2 * b]["out"]
        out[b, T // 2:] = results[2 * b + 1]["out"]
    return out


_CACHED = {}


def kernel(x, W_attn, b_attn, W_proj, b_proj):
    x = np.asarray(x, dtype=np.float32)
    W_attn = np.asarray(W_attn, dtype=np.float32)
    b_attn = np.asarray(b_attn, dtype=np.float32)
    W_proj = np.asarray(W_proj, dtype=np.float32)
    b_proj = np.asarray(b_proj, dtype=np.float32)
    T = x.shape[1]
    in_maps, wbq, wbp = shard_inputs(x, W_attn, b_attn, W_proj, b_proj)
    key = (T, wbq, wbp)
    if key not in _CACHED:
        _CACHED[key] = build_program(T, wbq, wbp)
    nc = _CACHED[key]
    res = run_bass_kernel_spmd(nc, in_maps, list(range(N_CORES)))
    return unshard_output(res.results, T)


# revision 21
# speedup vs baseline: 1.1087x; 1.0171x over previous
"""Causal self-attention (B=4, T=2048, C=768, H=12) on 8 trn2 NeuronCores.

Sharding: 4 batches x 2 head-groups = 8 cores. Each core computes the qkv
projection + attention for its 6 heads of one batch element in transposed
layout (q^T,k^T as [hd,T], v as [T,hd] -- zero on-device transposes), a
partial output projection over its 384 y-channels for all T, then a pairwise
ReduceScatter sums the two partial projections and hands each core one half
of the rows. All matmuls run in float32r (fast fp32, ~1.5e-4).

Host work is limited to slicing/transposing inputs and restacking outputs.
"""
import numpy as np
from contextlib import ExitStack

import concourse.bass as bass
import concourse.bacc as bacc
import concourse.mybir as mybir
import concourse.tile as tile
from concourse.bass_utils import run_bass_kernel_spmd

B, C, H, HD = 4, 768, 12, 64
N_CORES = 8
LH = H // 2          # local heads per core
F32 = mybir.dt.float32
F32R = mybir.dt.float32r
F16 = mybir.dt.float16
Exp = mybir.ActivationFunctionType.Exp
Copy = mybir.ActivationFunctionType.Copy
PAIRS = [[0, 1], [2, 3], [4, 5], [6, 7]]


def build_program(T=2048, with_bias_qkv=False, with_bias_proj=False, debug=False):
    CK = C // 128            # 6 contract chunks of the hidden dim
    QB = min(512, T)         # query block (free dim of S^T)
    NQB = T // QB
    NKC = T // 128           # key chunks
    DPB = QB // 128          # 128-wide diagonal strips per query block
    QKCOLS = 2 * LH * HD     # 768 local q+k columns
    VCOLS = LH * HD          # 384 local v columns
    WACOLS = QKCOLS + VCOLS  # 1152

    nc = bacc.Bacc("TRN2", target_bir_lowering=False, debug=False,
                   num_devices=N_CORES)
    xT_d = nc.dram_tensor("xT", [C, T], F32R, kind="ExternalInput")
    wa_d = nc.dram_tensor("wa", [C, WACOLS], F32R, kind="ExternalInput")
    wp_d = nc.dram_tensor("wp", [VCOLS, C], F32R, kind="ExternalInput")
    tri_d = nc.dram_tensor("tri", [128, 128], F32R, kind="ExternalInput")
    if with_bias_qkv:
        bq_d = nc.dram_tensor("bqkv", [1, WACOLS], F32R, kind="ExternalInput")
    if with_bias_proj:
        bp_d = nc.dram_tensor("bp", [1, C], F32R, kind="ExternalInput")
    out_d = nc.dram_tensor("out", [T // 2, C], F32, kind="ExternalOutput")
    if debug:
        dbg_v0 = nc.dram_tensor("dbg_v0", [128, LH * (HD + 1)], F32, kind="ExternalOutput")
        dbg_qkT0 = nc.dram_tensor("dbg_qkT0", [128, T], F32, kind="ExternalOutput")
        dbg_kT0 = nc.dram_tensor("dbg_kT0", [128, T], F32, kind="ExternalOutput")
        dbg_yps = nc.dram_tensor("dbg_yps", [HD + 1, 512], F32, kind="ExternalOutput")
        dbg_pt = nc.dram_tensor("dbg_pt", [128, 512], F32, kind="ExternalOutput")
        dbg_rec = nc.dram_tensor("dbg_rec", [1, 512], F32, kind="ExternalOutput")

    with tile.TileContext(nc) as tc, ExitStack() as top:
        persist = top.enter_context(tc.tile_pool(name="persist", bufs=1))
        dram = top.enter_context(tc.tile_pool(name="dram", bufs=1, space="DRAM"))

        # persistent tensors
        qkT = [persist.tile([128, T], F32R, tag=f"qkT{j}", name=f"qkT{j}") for j in range(CK)]
        kTs = [persist.tile([128, T], F32R, tag=f"kTs{j}", name=f"kTs{j}") for j in range(3)]
        yT = [persist.tile([128, T], F32R, tag=f"yT{j}", name=f"yT{j}") for j in range(3)]
        v_sb = [persist.tile([128, LH * (HD + 1)], F32R, tag=f"v{m}", name=f"v{m}")
                for m in range(NKC)]
        tri = persist.tile([128, 128], F32R, tag="tri")
        nc.sync.dma_start(tri[:], tri_d.ap())
        onescol = persist.tile([128, LH], F32R, tag="onescol")
        onescol_f = persist.tile([128, LH], F32, tag="onescol_f")
        nc.vector.memset(onescol_f[:], 1.0)
        nc.vector.tensor_copy(onescol[:], onescol_f[:])
        if with_bias_qkv:
            bq_sb = persist.tile([1, WACOLS], F32R, tag="bq")
            nc.sync.dma_start(bq_sb[:], bq_d.ap())
            onesq = persist.tile([1, QB], F32R, tag="onesq")
            onesq_f = persist.tile([1, QB], F32, tag="onesq_f")
            nc.vector.memset(onesq_f[:], 1.0)
            nc.vector.tensor_copy(onesq[:], onesq_f[:])
        if with_bias_proj:
            bp_sb = persist.tile([1, C], F32R, tag="bp")
            nc.sync.dma_start(bp_sb[:], bp_d.ap())
        if with_bias_qkv or with_bias_proj:
            ones128 = persist.tile([1, 128], F32R, tag="ones128")
            ones128_f = persist.tile([1, 128], F32, tag="ones128_f")
            nc.vector.memset(ones128_f[:], 1.0)
            nc.vector.tensor_copy(ones128[:], ones128_f[:])

        # ---------------- phase A: load + qkv projection ----------------
        with tc.tile_pool(name="phA", bufs=1) as phA, \
             tc.tile_pool(name="psA", bufs=2, space="PSUM") as psA:
            # warm the PE clock-gate while input DMAs stream
            for w in range(12):
                wps = psA.tile([128, 128], F32, tag="qps", name=f"warm{w}", bufs=4)
                nc.tensor.matmul(wps[:], tri[:], tri[:], start=True, stop=True)
            xt = [phA.tile([128, T], F32R, tag=f"xt{i}", name=f"xt{i}") for i in range(CK)]
            wa = [phA.tile([128, WACOLS], F32R, tag=f"wa{i}", name=f"wa{i}") for i in range(CK)]
            for i in range(CK):
                nc.sync.dma_start(xt[i][:], xT_d.ap()[128 * i:128 * (i + 1), :])
                nc.sync.dma_start(wa[i][:], wa_d.ap()[128 * i:128 * (i + 1), :])

            # Emission follows consumption order of phase B: for each query
            # block, its qk chains (+ per-slice kTs swaps) then its v chunks.
            def v_chain(m):
                vps = psA.tile([128, VCOLS], F32, tag="vps", name=f"vps{m}")
                for i in range(CK):
                    nc.tensor.matmul(
                        vps[:], xt[i][:, 128 * m:128 * (m + 1)],
                        wa[i][:, QKCOLS:WACOLS],
                        start=(i == 0),
                        stop=(i == CK - 1 and not with_bias_qkv))
                if with_bias_qkv:
                    nc.tensor.matmul(vps[:], ones128[:],
                                     bq_sb[:, QKCOLS:WACOLS],
                                     start=False, stop=True)
                nc.vector.tensor_copy(
                    v_sb[m][:].rearrange("p (h c) -> p h c", c=HD + 1)[:, :, 0:HD],
                    vps[:].rearrange("p (h c) -> p h c", c=HD))
                nc.vector.tensor_copy(
                    v_sb[m][:].rearrange("p (h c) -> p h c", c=HD + 1)[:, :, HD:HD + 1],
                    onescol[:].rearrange("p (h c) -> p h c", c=1))

            for n in range(NQB):
                for j in range(CK):
                    qps = psA.tile([128, QB], F32, tag="qps", name=f"qps{j}_{n}", bufs=4)
                    for i in range(CK):
                        nc.tensor.matmul(
                            qps[:], wa[i][:, 128 * j:128 * (j + 1)],
                            xt[i][:, QB * n:QB * (n + 1)],
                            start=(i == 0),
                            stop=(i == CK - 1 and not with_bias_qkv))
                    if with_bias_qkv:
                        nc.tensor.matmul(
                            qps[:], bq_sb[:, 128 * j:128 * (j + 1)], onesq[:],
                            start=False, stop=True)
                    nc.vector.tensor_copy(qkT[j][:, QB * n:QB * (n + 1)], qps[:])
                for m in range(DPB * n, DPB * (n + 1)):
                    v_chain(m)

        if debug:
            dv = persist.tile([128, LH * (HD + 1)], F32, tag="dv")
            nc.vector.tensor_copy(dv[:], v_sb[0][:])
            nc.sync.dma_start(dbg_v0.ap(), dv[:])
            dq = persist.tile([128, T], F32, tag="dq")
            nc.vector.tensor_copy(dq[:], qkT[0][:])
            nc.sync.dma_start(dbg_qkT0.ap(), dq[:])
            dk = persist.tile([128, T], F32, tag="dk")
            nc.vector.tensor_copy(dk[:], qkT[3][:])
            nc.sync.dma_start(dbg_kT0.ap(), dk[:])

        if debug:
            dv = persist.tile([128, LH * (HD + 1)], F32, tag="dv")
            nc.vector.tensor_copy(dv[:], v_sb[0][:])
            nc.sync.dma_start(dbg_v0.ap(), dv[:])
            dq = persist.tile([128, T], F32, tag="dq")
            nc.vector.tensor_copy(dq[:], qkT[0][:])
            nc.sync.dma_start(dbg_qkT0.ap(), dq[:])
            dk = persist.tile([128, T], F32, tag="dk")
            nc.vector.tensor_copy(dk[:], qkT[3][:])
            nc.sync.dma_start(dbg_kT0.ap(), dk[:])

        # half-swapped copies of k^T so both PE row groups can host any head
        for j in range(3):
            nc.sync.dma_start(kTs[j][64:128, :], qkT[3 + j][0:64, :])
            nc.sync.dma_start(kTs[j][0:64, :], qkT[3 + j][64:128, :])

        # ---------------- phase B+C interleaved ----------------
        # n-outer: once query block n is done for all heads, its four output
        # rows chunks are projected immediately; each half's ReduceScatter is
        # issued as soon as its projections are stored, hiding the collective
        # under the remaining attention work.
        # fp16 exchange: halves ReduceScatter bytes (~30us); partial values
        # are O(1) so fp16's 11-bit mantissa costs ~3e-4, no overflow risk.
        # Split in halves: the first RS fires ~40% into attention and can
        # overlap the remaining compute.
        SPLIT = 2 if NQB >= 2 else 1
        PH = T // SPLIT
        partials = [dram.tile([PH, C], F16, name=f"partial{i}") for i in range(SPLIT)]
        rs_outs = [dram.tile([PH // 2, C], F16, name=f"rs{i}") for i in range(SPLIT)]
        rs_done = [0] * SPLIT
        with tc.tile_pool(name="phB", bufs=6) as phB, \
             tc.tile_pool(name="phBs", bufs=2) as phBs, \
             tc.tile_pool(name="phC", bufs=1) as phC, \
             tc.tile_pool(name="stg", bufs=3) as stg, \
             tc.tile_pool(name="psS", bufs=3, space="PSUM") as psS, \
             tc.tile_pool(name="psY", bufs=2, space="PSUM") as psY, \
             tc.tile_pool(name="psC", bufs=1, space="PSUM") as psC:
            wp = [phC.tile([128, C], F32R, tag=f"wp{j}", name=f"wp{j}") for j in range(3)]
            for j in range(3):
                nc.sync.dma_start(wp[j][:], wp_d.ap()[128 * j:128 * (j + 1), :])

            for n in range(NQB):
                nkc = (QB // 128) * (n + 1)
                for h in range(LH):
                    jq, rq = h // 2, 64 * (h % 2)
                    klo = qkT[3 + jq] if h % 2 == 0 else kTs[jq]
                    khi = kTs[jq] if h % 2 == 0 else qkT[3 + jq]
                    yps = psY.tile([HD + 1, QB], F32, tag="yps")
                    # stage this (head, block)'s q at the opposite base
                    ob = 64 - rq
                    qst = phB.tile([128, QB], F32R, tag="qst", bufs=2)
                    nc.sync.dma_start(
                        qst[ob:ob + 64, :],
                        qkT[jq][rq:rq + 64, QB * n:QB * (n + 1)])
                    for kc0 in range(0, nkc, 2):
                        pair = [kc0] if kc0 + 1 >= nkc else [kc0, kc0 + 1]
                        # both S^T tiles of the pair land in one 2-bank psum
                        # tile; adjacent matmuls in distinct PE row groups run
                        # concurrently (K=64 row tiling)
                        spw = psS.tile([128, 2 * QB], F32, tag="sps", bufs=2)
                        ptw = phB.tile([128, 2 * QB], F32R, tag="pt", bufs=4)
                        offs = []
                        for pi, kc in enumerate(pair):
                            d = kc - DPB * n
                            c0 = 128 * d if d > 0 else 0
                            off = pi * QB
                            offs.append((kc, d, c0, off))
                            kt, rb = (klo, 0) if kc % 2 == 0 else (khi, 64)
                            if rb == rq:
                                qt_ap = qkT[jq][rq:rq + 64,
                                                QB * n + c0:QB * (n + 1)]
                            else:
                                qt_ap = qst[ob:ob + 64, c0:QB]
                            nc.tensor.matmul(
                                spw[:, off + c0:off + QB],
                                kt[rb:rb + 64, 128 * kc:128 * (kc + 1)],
                                qt_ap, start=True, stop=True)
                        if len(pair) == 2 and all(d < 0 for _, d, _, _ in offs):
                            nc.scalar.activation(ptw[:], spw[:], Exp, scale=0.125)
                        else:
                            for kc, d, c0, off in offs:
                                nc.scalar.activation(
                                    ptw[:, off + c0:off + QB],
                                    spw[:, off + c0:off + QB],
                                    Exp, scale=0.125)
                        for kc, d, c0, off in offs:
                            if d >= 0:
                                nc.vector.tensor_tensor(
                                    ptw[:, off + c0:off + c0 + 128],
                                    ptw[:, off + c0:off + c0 + 128],
                                    tri[:], mybir.AluOpType.mult)
                            if debug and h == 0 and n == 0 and kc == 0:
                                dpt = phB.tile([128, QB], F32, tag="dpt")
                                nc.vector.tensor_copy(dpt[:], ptw[:, 0:QB])
                                nc.sync.dma_start(dbg_pt.ap(), dpt[:, 0:512])
                        for kc, d, c0, off in offs:
                            nc.tensor.matmul(
                                yps[:, c0:QB],
                                v_sb[kc][:, (HD + 1) * h:(HD + 1) * (h + 1)],
                                ptw[:, off + c0:off + QB],
                                start=(kc == 0), stop=(kc == nkc - 1))
                    if debug and h == 0 and n == 0:
                        dyp = phBs.tile([HD + 1, QB], F32, tag="dyp")
                        nc.vector.tensor_copy(dyp[:], yps[:])
                        nc.sync.dma_start(dbg_yps.ap(), dyp[:, 0:512])
                    # normalize: yT[.] = yps[0:64] / yps[64]
                    s_sb = phBs.tile([1, QB], F32, tag="s_sb")
                    nc.vector.tensor_copy(s_sb[:], yps[HD:HD + 1, :])
                    rec = phBs.tile([1, QB], F32, tag="rec")
                    scr = phBs.tile([1, QB], F32, tag="scr")
                    nc.vector.reciprocal_approx_accurate(rec[:], s_sb[:], scr[:])
                    if debug and h == 0 and n == 0:
                        nc.sync.dma_start(dbg_rec.ap(), rec[:, 0:512])
                    recb = phBs.tile([64, QB], F32, tag="recb")
                    nc.gpsimd.partition_broadcast(recb[:], rec[:])
                    nc.vector.tensor_tensor(
                        yT[jq][rq:rq + 64, QB * n:QB * (n + 1)],
                        yps[0:HD, :], recb[:], mybir.AluOpType.mult)

                # project this query block's row chunks
                for m in range(DPB * n, DPB * (n + 1)):
                    pps = psC.tile([128, C], F32, tag="pp")
                    for ncol, c0, c1 in ((0, 0, 512), (1, 512, C)):
                        for j in range(3):
                            nc.tensor.matmul(
                                pps[:, c0:c1],
                                yT[j][:, 128 * m:128 * (m + 1)],
                                wp[j][:, c0:c1],
                                start=(j == 0),
                                stop=(j == 2 and not with_bias_proj))
                        if with_bias_proj:
                            nc.tensor.matmul(pps[:, c0:c1], ones128[:],
                                             bp_sb[:, c0:c1],
                                             start=False, stop=True)
                    ost = stg.tile([128, C], F16, tag="ost")
                    nc.scalar.activation(ost[:], pps[:], Copy)
                    hf = (m * 128) // PH
                    mh = m - hf * (PH // 128)
                    nc.sync.dma_start(
                        partials[hf][128 * mh:128 * (mh + 1), :], ost[:])
                    rs_done[hf] += 1
                    if rs_done[hf] == PH // 128:
                        nc.gpsimd.collective_compute(
                            "ReduceScatter", mybir.AluOpType.add,
                            replica_groups=PAIRS,
                            ins=[partials[hf].opt()],
                            outs=[rs_outs[hf].opt()])
                        for q2 in range(2):
                            r0 = q2 * PH // 4
                            r1 = (q2 + 1) * PH // 4
                            o0 = hf * PH // 2
                            nc.gpsimd.dma_start(
                                out=out_d.ap()[o0 + r0:o0 + r1, :],
                                in_=rs_outs[hf][r0:r1, :])
    nc.compile()
    return nc


def shard_inputs(x, W_attn, b_attn, W_proj, b_proj):
    """Per-core input maps. Core c = 2*b + g handles batch b, head-group g."""
    T = x.shape[1]
    tri = np.tril(np.ones((128, 128), dtype=np.float32)).T.copy()
    # tri[k_row, q_col] = 1 where k <= q  (lower-tri in (q,k) = upper in (k,q))
    with_bias_qkv = bool(np.any(b_attn))
    with_bias_proj = bool(np.any(b_proj))
    in_maps = []
    for c in range(N_CORES):
        b, g = divmod(c, 2)
        xT = np.ascontiguousarray(x[b].T)
        wq = W_attn[:, 384 * g:384 * (g + 1)]
        wk = W_attn[:, C + 384 * g:C + 384 * (g + 1)]
        wv = W_attn[:, 2 * C + 384 * g:2 * C + 384 * (g + 1)]
        wa = np.ascontiguousarray(np.concatenate([wq, wk, wv], axis=1))
        wp = np.ascontiguousarray(W_proj[384 * g:384 * (g + 1), :])
        m = {"xT": xT, "wa": wa, "wp": wp, "tri": tri}
        if with_bias_qkv:
            m["bqkv"] = np.concatenate(
                [b_attn[384 * g:384 * (g + 1)],
                 b_attn[C + 384 * g:C + 384 * (g + 1)],
                 b_attn[2 * C + 384 * g:2 * C + 384 * (g + 1)]]
            ).reshape(1, -1).astype(np.float32)
        if with_bias_proj:
            m["bp"] = (b_proj / 2.0).reshape(1, -1).astype(np.float32)
        in_maps.append(m)
    return in_maps, with_bias_qkv, with_bias_proj


def unshard_output(results, T):
    out = np.empty((B, T, C), dtype=np.float32)
    split = 2 if T >= 2 * min(512, T) else 1
    ph = T // split
    for b in range(B):
        for g in range(2):
            r = results[2 * b + g]["out"]
            for i in range(split):
                out[b, i * ph + g * ph // 2:i * ph + (g + 1) * ph // 2] = \
                    r[i * ph // 2:(i + 1) * ph // 2]
    return out


_CACHED = {}


def kernel(x, W_attn, b_attn, W_proj, b_proj):
    x = np.asarray(x, dtype=np.float32)
    W_attn = np.asarray(W_attn, dtype=np.float32)
    b_attn = np.asarray(b_attn, dtype=np.float32)
    W_proj = np.asarray(W_proj, dtype=np.float32)
    b_proj = np.asarray(b_proj, dtype=np.float32)
    T = x.shape[1]
    in_maps, wbq, wbp = shard_inputs(x, W_attn, b_attn, W_proj, b_proj)
    key = (T, wbq, wbp)
    if key not in _CACHED:
        _CACHED[key] = build_program(T, wbq, wbp)
    nc = _CACHED[key]
    res = run_bass_kernel_spmd(nc, in_maps, list(range(N_CORES)))
    return unshard_output(res.results, T)
